# revision 1
# baseline (speedup 1.0000x reference)
"""NetVLAD pooling kernel for Trainium2 (Bass/Tile), 8-core data-parallel.

Reference computation (per batch b):
    scores = conv_w @ x[b]                  # [K, N]
    assign = softmax(scores, axis=K)
    vlad   = x[b] @ assign.T - centers * assign.sum(n)   # [D, K]
    vlad  /= max(||vlad||_2 over D, eps)    # intra-norm per cluster column
    desc   = vlad.reshape(D*K) / max(||.||_2, eps)

Shapes: x [32, 512, 1024] f32, conv_w [64, 512], centers [512, 64],
output desc [32, 32768] f32.  Sharding: data-parallel over batch,
4 batches per core; params replicated.

Layout strategy per core (all matmul inputs bf16, PSUM accum f32):
  * x ships in bf16 natural layout [d, n] (halves HBM bytes vs f32).
    The vlad contraction needs x^T [n, d]: for the first NSHIP batches
    x^T is also shipped (in 4 chunks each, so the vlad matmuls chase
    the DMA), and for the remaining batches x^T is produced ON-CHIP by
    PE transposes (bf16 = 1 cyc/row) whose PSUM staging is drained to
    per-chunk SBUF tiles by DVE/ACT (the only engines that can read
    PSUM) — keeping the serialized DMA stream ~25% shorter and every
    engine busy during it.
  * scores are computed directly TRANSPOSED, [n, k] (stationary = x
    chunk, moving = conv_w^T, 64 cols): softmax over clusters becomes a
    free-dim reduce on DVE, and the normalized assignment AN [n, k] is
    exactly the stationary operand the vlad matmul needs.  Max-
    subtraction is dropped (scores ~ N(0,1): exp cannot overflow).
  * vlad^T [k, d] accumulates over the 8 n-chunks; the assign row-sums
    ride along as a 2-column ones-matmul reusing the same stationary.
  * Tail is fused and engine-balanced: centers^T is shipped NEGATED,
    so V = (-c^T * asum) + vlad^T is ONE DVE op reading asum straight
    from PSUM; ||V||^2 via ACT Square+accum_out; the global 1/8 norm
    folds into ACT Sqrt(rss/64); Vn = V*rinv8 is an ACT Copy with a
    per-partition scale AP (moved to DVE for the batches whose chains
    would otherwise serialize the ACT queue, so the three final desc
    DMAs don't pile up on the serial DMA engine).  Only Exp / Sqrt /
    Copy / Square run on ACT, in phase order: exactly 2 table loads.
  * The LAST batch's PSUM accumulator and asum tiles scavenge rings
    that are already dead by then (the scores bank after the final
    exp, a gen-staging bank after the last x^T copy), and every tail
    SBUF tag holds one slot per batch — so no accumulation or tail op
    ever waits on another batch's reader to free a buffer.
  * Emission order matches data arrival so the in-order engine queues
    never park ready work behind gated work; per-batch tails fire as
    soon as their own accumulation stops, and output transposes are
    slotted where the PE would otherwise idle.
  * The second L2 normalization is a constant 1/8: after intra-norm
    each of the K=64 columns has unit norm, so ||desc|| = 8.
"""

import numpy as np
import ml_dtypes

import concourse.bass as bass
from concourse import bacc
import concourse.mybir as mybir
import concourse.tile as tile
from concourse.bass_utils import run_bass_kernel_spmd
from concourse.masks import make_identity

B, D, K, N = 32, 512, 64, 1024
NCORES = 8
BC = B // NCORES          # batches per core
DC = D // 128             # d chunks (4)
NB = N // 128             # n chunks (8)
NH = 4                    # xt DMA chunks per shipped batch
NSHIP = 2                 # batches whose x^T ships via DMA (rest: PE-gen)
F32 = mybir.dt.float32
BF16 = mybir.dt.bfloat16
NP_BF16 = np.dtype(ml_dtypes.bfloat16)
AF = mybir.ActivationFunctionType


def _netvlad_core(ctx, tc, out, xn, xt, w, c):
    """Emit the per-core tile program.

    out: desc [BC, D*K] f32 DRAM
    xn:  x natural  [BC, 128, DC, N] bf16 DRAM   (p=d%128, cc=d//128)
    xt:  x^T of batches 0..NSHIP-1 [NSHIP, NH, 128, NB//NH, D] bf16
    w:   conv_w^T  [128, DC, K] bf16 DRAM
    c:   NEGATED centers^T [2K, DC, 128] f32 DRAM (replicated halves)
    """
    nc = tc.nc

    const = ctx.enter_context(tc.tile_pool(name="const", bufs=1))
    xnp = ctx.enter_context(tc.tile_pool(name="xnp", bufs=1))
    xtp = ctx.enter_context(tc.tile_pool(name="xtp", bufs=1))
    xgp = ctx.enter_context(tc.tile_pool(name="xgp", bufs=1))
    epool = ctx.enter_context(tc.tile_pool(name="ep", bufs=2))
    apool = ctx.enter_context(tc.tile_pool(name="ap", bufs=BC))
    vpool = ctx.enter_context(tc.tile_pool(name="vp", bufs=2))
    opool = ctx.enter_context(tc.tile_pool(name="op", bufs=2))
    spool = ctx.enter_context(tc.tile_pool(name="sp", bufs=4))
    # PSUM 8 banks: s(1) + v(2) + as(2) + o(1) + stage(2)
    ps_s = ctx.enter_context(tc.tile_pool(name="pss", bufs=1, space="PSUM"))
    ps_v = ctx.enter_context(tc.tile_pool(name="psv", bufs=2, space="PSUM"))
    ps_a = ctx.enter_context(tc.tile_pool(name="psa", bufs=2, space="PSUM"))
    ps_o = ctx.enter_context(tc.tile_pool(name="pso", bufs=1, space="PSUM"))
    ps_g = ctx.enter_context(tc.tile_pool(name="psg", bufs=2, space="PSUM"))

    # ---- params + x loads --------------------------------------------
    wT = const.tile([128, DC, K], BF16, tag="wT")
    nc.sync.dma_start(wT, w)
    cTn = const.tile([128, DC, 128], F32, tag="cTn")
    ident = const.tile([128, 128], BF16, tag="ident")
    make_identity(nc, ident)
    ones2 = const.tile([128, 2], BF16, tag="ones2")
    nc.vector.memset(ones2, 1.0)
    # touch Exp immediately so the 1.3us act-table load overlaps the DMAs
    warm = const.tile([1, 2], F32, tag="warm")
    nc.scalar.activation(warm, ones2[0:1, :], func=AF.Exp)

    xns = []
    for b in range(BC):
        halves = []
        for h in range(2):
            xh = xnp.tile([128, DC, N // 2], BF16, tag=f"xn{h}",
                          name=f"xn{b}_{h}", bufs=BC)
            nc.sync.dma_start(xh, xn[b, :, :, h * (N // 2):(h + 1) * (N // 2)])
            halves.append(xh)
        xns.append(halves)
    nc.sync.dma_start(cTn, c)
    xt_ship = []
    for b in range(NSHIP):
        tb = []
        for h in range(NH):
            th = xtp.tile([128, NB // NH, D], BF16, tag=f"xt{h}",
                          name=f"xt{b}_{h}", bufs=NSHIP)
            nc.sync.dma_start(th, xt[b, h])
            tb.append(th)
        xt_ship.append(tb)

    cTf = cTn.rearrange("p cc d -> p (cc d)")
    desc_v = out.rearrange("b (cc p k) -> p cc b k", cc=DC, p=128, k=K)

    # ---- phase 1 (scores^T -> exp -> softmax) + x^T generation --------
    # PE emission order: scores(0), scores(1), gen(1), scores(2), gen(2),
    # ... — gen transposes of batch b fill the PE while batch b+1's xn
    # is still arriving.
    # PSUM can only be read by DVE and ACT on real hardware (GPSIMD is
    # SBUF-only), so every PSUM-drain copy alternates between those two.
    copy_engines = [nc.vector, nc.scalar, nc.vector, nc.scalar,
                    nc.vector, nc.scalar, nc.vector, nc.scalar]
    ANs, xt_gen = [], {}

    def xslice(b, cc, nj):
        # n-halved tiles: scores/gen for n-chunk nj only need half nj//4,
        # so batch b's phase 1 starts when the first half of its xn lands
        h, j = nj // (NB // 2), nj % (NB // 2)
        return xns[b][h][:, cc, j * 128:(j + 1) * 128]

    def emit_scores(b):
        s_ps = ps_s.tile([128, NB, K], F32, tag="s", name=f"s{b}")
        for nj in range(NB):
            for cc in range(DC):
                nc.tensor.matmul(
                    s_ps[:, nj, :],
                    lhsT=xslice(b, cc, nj),
                    rhs=wT[:, cc, :],
                    start=(cc == 0),
                    stop=(cc == DC - 1),
                )
        E = epool.tile([128, NB, K], F32, tag="E", name=f"E{b}")
        nc.scalar.activation(E, s_ps, func=AF.Exp)

        red = spool.tile([128, NB], F32, tag="red", name=f"red{b}")
        nc.vector.tensor_reduce(
            red, E, axis=mybir.AxisListType.X, op=mybir.AluOpType.add
        )
        rec = spool.tile([128, NB], F32, tag="rec", name=f"rec{b}")
        nc.vector.reciprocal(rec, red)
        AN = apool.tile([128, NB, K], BF16, tag="AN", name=f"AN{b}")
        rec_b = bass.AP(
            tensor=rec.tensor,
            offset=rec.offset,
            ap=[rec.ap[0], [1, NB], [0, K]],
        )
        nc.vector.tensor_mul(AN, E, rec_b)
        ANs.append(AN)

    def emit_gen(b):
        """x^T of batch b via PE transposes.  Each n-chunk gets its OWN
        SBUF tile: the tile framework tracks dependencies per tile, so a
        single shared x^T tile would serialize all 8 PSUM-drain copies
        (cross-engine, a sem hop each) no matter which engines run them."""
        tiles = []
        for nj in range(NB):
            g_ps = ps_g.tile([128, DC, 128], BF16, tag="g", name=f"g{b}_{nj}")
            for cc in range(DC):
                nc.tensor.transpose(g_ps[:, cc, :], xslice(b, cc, nj), ident)
            xgt = xgp.tile([128, D], BF16, tag=f"xg{nj}", name=f"xg{b}_{nj}",
                           bufs=BC - NSHIP)
            eng = copy_engines[nj % len(copy_engines)]
            if eng is nc.scalar:
                nc.scalar.copy(xgt, g_ps)
            else:
                eng.tensor_copy(xgt, g_ps)
            tiles.append(xgt)
        xt_gen[b] = tiles

    # ---- phase 2: vlad^T + asum -> per-batch tails --------------------
    # Per-batch PSUM banks (a shared col-packed bank looks clever but the
    # tile framework tracks deps per tile, so one batch's tail READ of
    # the bank serializes the other batch's matmuls behind it).
    # Emission follows data arrival: b2 (x^T generated early), b3 (x^T
    # copies trickling in) woven with b0 (DMA chunks trickling in), then
    # b1 (last DMA chunks).  Tails fire per batch as soon as their own
    # accumulation stops; output transposes are slotted where the PE
    # would otherwise idle.

    def vlad_rhs(b, nj):
        if b < NSHIP:
            th = xt_ship[b][nj // (NB // NH)]
            return th[:, nj % (NB // NH), :]
        return xt_gen[b][nj]

    tails = {}

    def vlad_mms(b, ring):
        # two independent single-buffer rings per resource: a late batch
        # reuses the slot freed by the EARLIEST finished tail, not the
        # most recent one.  The LAST batch (ring < 0) scavenges banks
        # that are already dead by then: the scores bank (idle after the
        # final exp) and a gen-staging bank (idle after the last x^T
        # copy) — so its accumulation never waits on another tail's read.
        if ring < 0:
            v_ps = ps_s.tile([64, D], F32, tag="s", name=f"v{b}")
            as_ps = ps_g.tile([64, 2], F32, tag="g", name=f"as{b}")
        else:
            v_ps = ps_v.tile([64, D], F32, tag=f"v{ring}", name=f"v{b}",
                             bufs=1)
            as_ps = ps_a.tile([64, 2], F32, tag=f"as{ring}", name=f"as{b}",
                              bufs=1)
        tails[b] = (v_ps, as_ps)
        for nj in range(NB):
            nc.tensor.matmul(
                v_ps,
                lhsT=ANs[b][:, nj, :],
                rhs=vlad_rhs(b, nj),
                start=(nj == 0),
                stop=(nj == NB - 1),
            )
            nc.tensor.matmul(
                as_ps,
                lhsT=ANs[b][:, nj, :],
                rhs=ones2,
                start=(nj == 0),
                stop=(nj == NB - 1),
            )
            yield

    def tail_math(b, last=False):
        """V -> ||V||^2 -> 1/ss -> sqrt -> Vn, per batch.
        (tensor_tensor_reduce compiles but crashes the runtime on this
        stack.)  Mid-stream batches square+accumulate on ACT (one op,
        engine-seconds cheap); the last batch keeps the whole chain on
        DVE to avoid two cross-engine sem hops on the critical path."""
        v_ps, as_ps = tails[b]
        V = vpool.tile([64, D], F32, tag="V", name=f"V{b}", bufs=BC)
        nc.vector.scalar_tensor_tensor(
            V, cTf[0:64, :], as_ps[:, 0:1], v_ps,
            op0=mybir.AluOpType.mult, op1=mybir.AluOpType.add,
        )
        # squares all on ACT (its tail queue has a natural slot for each
        # batch); Vn of the second-to-last batch on DVE, which is idle
        # after its own chain — balances ~9 us of tail work across both
        # PSUM-capable engines so the three final desc DMAs stop piling up
        sq = vpool.tile([64, D], F32, tag="sq", name=f"sq{b}", bufs=BC)
        ss = spool.tile([64, 1], F32, tag="ss", name=f"ss{b}")
        nc.scalar.activation(sq, V, func=AF.Square, accum_out=ss)
        rss = spool.tile([64, 1], F32, tag="rss", name=f"rss{b}")
        nc.vector.reciprocal(rss, ss)
        r8 = spool.tile([64, 1], F32, tag="r8", name=f"r8{b}")
        nc.scalar.activation(r8, rss, func=AF.Sqrt, scale=1.0 / 64.0)
        Vn = vpool.tile([64, D], BF16, tag="Vn", name=f"Vn{b}", bufs=BC)
        if last:
            nc.vector.tensor_scalar_mul(Vn, V, r8)
        else:
            nc.scalar.activation(Vn, V, func=AF.Copy, scale=r8)
        tails[b] = Vn

    def tail_out(b, eng):
        Vn = tails[b]
        o_ps = ps_o.tile([128, DC, K], BF16, tag="o", name=f"o{b}")
        for cc in range(DC):
            nc.tensor.transpose(
                o_ps[:, cc, :], Vn[:, cc * 128:(cc + 1) * 128], ident[:64, :64]
            )
        o_sb = opool.tile([128, DC, K], F32, tag="O", name=f"O{b}", bufs=BC)
        if eng is nc.scalar:
            nc.scalar.copy(o_sb, o_ps)
        else:
            eng.tensor_copy(o_sb, o_ps)
        nc.sync.dma_start(desc_v[:, :, b, :], o_sb)

    emit_scores(0)
    emit_scores(1)
    emit_scores(2)
    emit_gen(2)
    emit_scores(3)
    emit_gen(3)
    for _ in vlad_mms(2, 0):
        pass
    tail_math(2)
    g0, g3 = vlad_mms(0, 1), vlad_mms(3, 0)
    for nj in range(NB):
        next(g0, None)
        next(g3, None)
    next(g0, None)
    next(g3, None)
    tail_math(0)
    tail_math(3, last=True)
    tail_out(2, nc.vector)
    g1 = vlad_mms(1, -1)
    for nj in range(6):
        next(g1, None)
    tail_out(3, nc.vector)
    tail_out(0, nc.vector)
    next(g1, None)
    next(g1, None)
    tail_math(1)
    tail_out(1, nc.scalar)


_NC_CACHE = None


def _build_nc():
    global _NC_CACHE
    if _NC_CACHE is not None:
        return _NC_CACHE
    from contextlib import ExitStack

    nc = bacc.Bacc("TRN2", target_bir_lowering=False, debug=False,
                   num_devices=NCORES)
    xn = nc.dram_tensor("xn", [BC, 128, DC, N], BF16, kind="ExternalInput").ap()
    xt = nc.dram_tensor("xt", [NSHIP, NH, 128, NB // NH, D], BF16,
                        kind="ExternalInput").ap()
    w = nc.dram_tensor("wt", [128, DC, K], BF16, kind="ExternalInput").ap()
    c = nc.dram_tensor("ct", [2 * K, DC, 128], F32, kind="ExternalInput").ap()
    out = nc.dram_tensor("desc", [BC, D * K], F32, kind="ExternalOutput").ap()
    with tile.TileContext(nc) as tc, ExitStack() as ctx:
        _netvlad_core(ctx, tc, out, xn, xt, w, c)
    nc.compile()
    _NC_CACHE = nc
    return nc


def _prep_inputs(x, conv_w, centers):
    """Host-side sharding + layout prep (bf16 cast, pre-transpose)."""
    wt = np.ascontiguousarray(
        conv_w.T.reshape(DC, 128, K).transpose(1, 0, 2)
    ).astype(NP_BF16)
    ct1 = -centers.T.reshape(K, DC, 128)
    ct = np.ascontiguousarray(np.concatenate([ct1, ct1], axis=0))
    in_maps = []
    for i in range(NCORES):
        xc = x[i * BC:(i + 1) * BC]
        xn = np.ascontiguousarray(
            xc.reshape(BC, DC, 128, N).transpose(0, 2, 1, 3)
        ).astype(NP_BF16)
        # [b, n, d] -> [b, h, p, j, d] with n = (h*(NB//NH) + j)*128 + p
        xt = np.ascontiguousarray(
            xc[:NSHIP].transpose(0, 2, 1)
            .reshape(NSHIP, NH, NB // NH, 128, D)
            .transpose(0, 1, 3, 2, 4)
        ).astype(NP_BF16)
        in_maps.append({"xn": xn, "xt": xt, "wt": wt, "ct": ct})
    return in_maps


def kernel(x, conv_w, centers):
    x = np.ascontiguousarray(x, dtype=np.float32)
    conv_w = np.ascontiguousarray(conv_w, dtype=np.float32)
    centers = np.ascontiguousarray(centers, dtype=np.float32)
    nc = _build_nc()
    in_maps = _prep_inputs(x, conv_w, centers)
    res = run_bass_kernel_spmd(nc, in_maps, core_ids=list(range(NCORES)))
    return np.concatenate([r["desc"] for r in res.results], axis=0)



# revision 7
# speedup vs baseline: 1.1894x; 1.1894x over previous
"""NetVLAD pooling kernel for Trainium2 (Bass/Tile), 8-core data-parallel.

Reference computation (per batch b):
    scores = conv_w @ x[b]                  # [K, N]
    assign = softmax(scores, axis=K)
    vlad   = x[b] @ assign.T - centers * assign.sum(n)   # [D, K]
    vlad  /= max(||vlad||_2 over D, eps)    # intra-norm per cluster column
    desc   = vlad.reshape(D*K) / max(||.||_2, eps)

Shapes: x [32, 512, 1024] f32, conv_w [64, 512], centers [512, 64],
output desc [32, 32768] f32.  Sharding: data-parallel over batch,
4 batches per core; params replicated.

v2 strategy (everything fp8 e4m3 on the PE, DoubleRow perf mode):
  * x ships once in fp8 natural layout (2 MB/core, half of bf16), with
    the d axis PERMUTED as d(p, cc) = 4p + cc.  The same permutation is
    applied to conv_w and centers host-side, so scores contract
    identically, and the vlad output tile [128p, 4cc, 64k] maps to
    CONTIGUOUS 1 KB runs of the desc row -> full-bandwidth output DMA.
  * scores^T [n, k] via DoubleRow matmuls (contraction 256/instr,
    0.5 cyc/col): 512 PE cycles per batch.  Softmax over k is a
    free-dim reduce; exp cannot overflow (scores ~ N(0,1)), max
    subtraction dropped.  E in bf16, AN in fp8.
  * x^T is GENERATED on chip (never shipped): a non-transpose DoubleRow
    matmul against a block-diagonal fp8 identity [128, 2, 256] yields
    TWO transposed d-chunks per instruction (128 cyc for 256 cols).
    PSUM staging [128, 2, 256] f32 is drained to fp8 stage tiles by
    DVE/ACT (the only PSUM-capable engines); the drain volume (16K
    engine-cycles total) is the binding engine constraint and is split
    between both engines.
  * vlad [d, k] via DoubleRow (stage as stationary, AN moving):
    512 cycles per batch.  The centers term is folded into the SAME
    PSUM accumulation as one bf16 matmul per d-chunk against
    diag(asum) (built by DVE as ident64 * asum); asum itself rides as
    a tiny DoubleRow ones-matmul.
  * intra-norm without transposes: ACT Square drains V -> Vsq bf16
    (table-compatible with Exp), PE ones-column matmuls accumulate
    ssq as a ROW [1, 64], DVE reciprocal, ACT Sqrt(q/64) folds the
    global 1/8 norm (each of the 64 unit columns contributes 1), a
    1-partition ones matmul broadcasts rinv to [128, 64], and one DVE
    multiply reads V straight from PSUM to produce the f32 output.
  * ACT runs exactly two table loads: {Exp, Copy, Square} phase, then
    ALL Sqrts deferred until after the last batch's Exp.
  * PE warms up on dummy DoubleRow matmuls during the initial DMA so
    the p-state ramp completes before real work arrives.
"""

import numpy as np
import ml_dtypes

import concourse.bass as bass
from concourse import bacc
import concourse.mybir as mybir
import concourse.tile as tile
from concourse.bass_utils import run_bass_kernel_spmd
from concourse.masks import make_identity

B, D, K, N = 32, 512, 64, 1024
NCORES = 8
BC = B // NCORES          # batches per core (4)
DC = D // 128             # d chunks (4)
NB = N // 128             # n chunks (8)
NJ = NB // 2              # double-n-chunks (4)
F32 = mybir.dt.float32
BF16 = mybir.dt.bfloat16
FP8 = mybir.dt.float8e4
NP_FP8 = np.dtype(ml_dtypes.float8_e4m3)
NP_BF16 = np.dtype(ml_dtypes.bfloat16)
AF = mybir.ActivationFunctionType
DR = mybir.MatmulPerfMode.DoubleRow


def _netvlad_core(ctx, tc, out, xn, w, c):
    """Emit the per-core tile program.

    out: desc [BC, D*K] f32 DRAM
    xn:  x permuted [BC, 128, DC, N] fp8   (xn[b,p,cc,n] = x[b, 4p+cc, n])
    w:   conv_w^T  [128, DC, K] fp8        (w[p,cc,k] = conv_w[k, 4p+cc])
    c:   NEGATED centers^T [K, DC, 128] bf16 (c[k,cc,i] = -centers[4i+cc, k])
    """
    nc = tc.nc

    const = ctx.enter_context(tc.tile_pool(name="const", bufs=1))
    xpool = ctx.enter_context(tc.tile_pool(name="xp", bufs=1))
    epool = ctx.enter_context(tc.tile_pool(name="ep", bufs=2))
    apool = ctx.enter_context(tc.tile_pool(name="ap", bufs=BC))
    spool = ctx.enter_context(tc.tile_pool(name="sp", bufs=2))
    stpool = ctx.enter_context(tc.tile_pool(name="st", bufs=2))
    vpool = ctx.enter_context(tc.tile_pool(name="vp", bufs=2))
    opool = ctx.enter_context(tc.tile_pool(name="op", bufs=BC))
    # PSUM: s(2) + g(3) + v(2, smalls packed into the v bank) -> 7 banks
    ps_s = ctx.enter_context(tc.tile_pool(name="pss", bufs=2, space="PSUM"))
    ps_g = ctx.enter_context(tc.tile_pool(name="psg", bufs=3, space="PSUM"))
    ps_v = ctx.enter_context(tc.tile_pool(name="psv", bufs=2, space="PSUM"))

    # ---- params + constants ------------------------------------------
    wT = const.tile([128, DC, K], FP8, tag="wT")
    nc.sync.dma_start(wT, w)
    cT = const.tile([K, DC, 128], BF16, tag="cT")
    nc.sync.dma_start(cT, c)

    # block-diagonal fp8 identity for the DoubleRow transpose-matmul
    ibig = const.tile([128, 2, 256], FP8, tag="ibig")
    nc.gpsimd.memset(ibig, 0.0)
    make_identity(nc, ibig[:, 0, 0:128], nomemset=True)
    make_identity(nc, ibig[:, 1, 128:256], nomemset=True)
    id64 = const.tile([64, 64], BF16, tag="id64")
    make_identity(nc, id64)
    ones_col = const.tile([128, 1], BF16, tag="ones_col")
    nc.vector.memset(ones_col, 1.0)
    ones_row = const.tile([1, 128], BF16, tag="ones_row")
    nc.vector.memset(ones_row, 1.0)
    ones2 = const.tile([128, 2, 2], FP8, tag="ones2")
    nc.vector.memset(ones2, 1.0)
    # touch Exp immediately so the 1.3us act-table load overlaps the DMAs
    warm = const.tile([1, 2], F32, tag="warm")
    nc.scalar.activation(warm, ones_row[0:1, 0:2], func=AF.Exp)

    # ---- x loads ------------------------------------------------------
    # b0 halves (early start), b1/b2 whole, b3 quarters (tail chase)
    xns = []
    for b in range(BC):
        xt = xpool.tile([128, DC, N], FP8, tag="xn", name=f"xn{b}", bufs=BC)
        xns.append(xt)
    nchunks = {0: 2, 1: 1, 2: 1, 3: 4}
    for b in range(BC):
        nch = nchunks[b]
        w_ = N // nch
        for h in range(nch):
            nc.sync.dma_start(
                xns[b][:, :, h * w_:(h + 1) * w_],
                xn[b, :, :, h * w_:(h + 1) * w_],
            )

    desc_v = out.rearrange("b (p cc k) -> p b cc k", p=128, cc=DC, k=K)

    # ---- PE warmup: dummy DoubleRow matmuls during the DMA lead-in ----
    for i in range(20):
        gd = ps_g.tile([128, 2, 256], F32, tag="g", name=f"warmmm{i}")
        nc.tensor.matmul(gd[:, 0, :], lhsT=ibig[:, :, 0:128], rhs=ibig,
                         start=True, stop=True, perf_mode=DR)

    # ---- per-batch pieces --------------------------------------------
    s_tiles, Es, ANs, reds, recs = {}, {}, {}, {}, {}
    stages = {}       # (b, J, P) -> stage tile
    v_tiles = {}
    as_tiles = {}
    vsqs, qrows, rinvs, rbs = {}, {}, {}, {}

    def scores_mms(b, njs):
        # ONE start=True per PSUM bank per batch: on this stack a start
        # marks the whole 2KB bank pending-zero, so a second start would
        # invalidate other regions' already-written bytes.  start=False
        # first-writes see pending-zero and overwrite (not accumulate).
        if b not in s_tiles:
            s_tiles[b] = (ps_s.tile([128, NB, K], F32, tag="s", name=f"s{b}"),
                          [True])
        s_ps, fresh = s_tiles[b]
        for nj in njs:
            for t in range(2):
                nc.tensor.matmul(
                    s_ps[:, nj, :],
                    lhsT=xns[b][:, 2 * t:2 * t + 2, nj * 128:(nj + 1) * 128],
                    rhs=wT[:, 2 * t:2 * t + 2, :],
                    start=fresh[0], stop=(t == 1), perf_mode=DR,
                    skip_group_check=True,
                )
                fresh[0] = False
        return s_ps

    def gen_mms(b, J, P):
        """x^T gen for double-chunk J, cc-pair P. Returns PSUM tile."""
        g = ps_g.tile([128, 2, 256], F32, tag="g", name=f"g{b}_{J}_{P}")
        for tp in range(2):          # tp: which nj of the pair
            nj = 2 * J + tp
            nc.tensor.matmul(
                g[:, tp, :],
                lhsT=xns[b][:, 2 * P:2 * P + 2, nj * 128:(nj + 1) * 128],
                rhs=ibig,
                start=True, stop=True, perf_mode=DR,
            )
        return g

    def gen_drain(b, J, P, g, eng):
        st = stpool.tile([128, 2, 2, 128], FP8, tag=f"S{J}{P}",
                         name=f"S{b}_{J}_{P}")
        if eng is nc.scalar:
            nc.scalar.copy(st, g)
        else:
            nc.vector.tensor_copy(st, g)
        stages[(b, J, P)] = st

    def exp_op(b, njs, name):
        s_ps = s_tiles[b][0]
        if b not in Es:
            Es[b] = epool.tile([128, NB, K], BF16, tag="E", name=f"E{b}")
        E = Es[b]
        nj0, nj1 = njs[0], njs[-1] + 1
        nc.scalar.activation(E[:, nj0:nj1, :], s_ps[:, nj0:nj1, :],
                             func=AF.Exp)
        return E

    def softmax_tail(b, njs, mul_eng):
        """reduce + reciprocal + AN mul for n-chunks njs."""
        E = Es[b]
        if b not in reds:
            reds[b] = spool.tile([128, NB], F32, tag="red", name=f"red{b}")
            recs[b] = spool.tile([128, NB], F32, tag="rec", name=f"rec{b}")
            ANs[b] = apool.tile([128, NB, K], FP8, tag="AN", name=f"AN{b}")
        red, rec, AN = reds[b], recs[b], ANs[b]
        nj0, nj1 = njs[0], njs[-1] + 1
        nc.vector.tensor_reduce(
            red[:, nj0:nj1], E[:, nj0:nj1, :],
            axis=mybir.AxisListType.X, op=mybir.AluOpType.add,
        )
        nc.vector.reciprocal(rec[:, nj0:nj1], red[:, nj0:nj1])
        rec_stride = rec.ap[-1][0]
        rec_b = bass.AP(
            tensor=rec.tensor,
            offset=rec.offset + nj0 * rec_stride,
            ap=[rec.ap[0], [rec_stride, nj1 - nj0], [0, K]],
        )
        mul_eng.tensor_mul(AN[:, nj0:nj1, :], E[:, nj0:nj1, :], rec_b)

    def vlad_mms(b, J, first, last):
        if b not in v_tiles:
            vb = ps_v.tile([128, 512], F32, tag="v", name=f"v{b}")
            v_tiles[b] = vb
            as_tiles[b] = vb[0:64, 256:258]
        v_ps = v_tiles[b][:, 0:256].rearrange("p (cc k) -> p cc k", cc=DC)
        as_ps = as_tiles[b]
        AN = ANs[b]
        rhs = AN[:, 2 * J:2 * J + 2, :]
        for cc in range(DC):
            nc.tensor.matmul(
                v_ps[:, cc, :],
                lhsT=stages[(b, J, cc // 2)][:, :, cc % 2, :],
                rhs=rhs,
                start=(first and cc == 0), stop=False, perf_mode=DR,
                skip_group_check=True,
            )
        nc.tensor.matmul(
            as_ps, lhsT=rhs, rhs=ones2,
            start=False, stop=last, perf_mode=DR,
            skip_group_check=True,
        )

    def centers_mms(b):
        """diag(asum) build + fold centers into the v accumulation."""
        v_ps = v_tiles[b][:, 0:256].rearrange("p (cc k) -> p cc k", cc=DC)
        as_ps = as_tiles[b]
        diag = spool.tile([64, 64], BF16, tag="diag", name=f"diag{b}")
        nc.vector.tensor_scalar_mul(diag, id64, as_ps[:, 0:1])
        for cc in range(DC):
            nc.tensor.matmul(
                v_ps[:, cc, :], lhsT=cT[:, cc, :], rhs=diag,
                start=False, stop=True, skip_group_check=True,
            )

    def tail_a(b):
        """Square-drain V, column-sum ssq row, reciprocal."""
        v_ps = v_tiles[b][:, 0:256].rearrange("p (cc k) -> p cc k", cc=DC)
        vsq = vpool.tile([128, DC, K], BF16, tag="vsq", name=f"vsq{b}")
        nc.scalar.activation(vsq, v_ps, func=AF.Square)
        vsqs[b] = vsq
        ssq = v_tiles[b][0:1, 320:384]
        for cc in range(DC):
            nc.tensor.matmul(
                ssq, lhsT=ones_col, rhs=vsq[:, cc, :],
                start=False, stop=(cc == DC - 1), skip_group_check=True,
            )
        q = spool.tile([1, K], F32, tag="q", name=f"q{b}")
        nc.vector.reciprocal(q, ssq)
        qrows[b] = q

    def tail_sqrt(b):
        """rinv8 = sqrt(q/64) bf16 (sqrt-table phase)."""
        r = spool.tile([1, K], BF16, tag="rinv", name=f"rinv{b}", bufs=BC)
        nc.scalar.activation(r, qrows[b], func=AF.Sqrt, scale=1.0 / 64.0)
        rinvs[b] = r

    def tail_b(b, out_eng):
        """broadcast rinv, final scale, output DMA."""
        v_ps = v_tiles[b][:, 0:256].rearrange("p (cc k) -> p cc k", cc=DC)
        rb_ps = v_tiles[b][:, 384:448]
        nc.tensor.matmul(rb_ps, lhsT=ones_row, rhs=rinvs[b],
                         start=False, stop=True, skip_group_check=True)
        rb = spool.tile([128, K], F32, tag="rb", name=f"rbs{b}")
        if out_eng is nc.scalar:
            nc.scalar.copy(rb, rb_ps)
        else:
            nc.vector.tensor_copy(rb, rb_ps)
        rb_b = bass.AP(tensor=rb.tensor, offset=rb.offset,
                       ap=[rb.ap[0], [0, DC], [1, K]])
        vn = opool.tile([128, DC, K], F32, tag="vn", name=f"vn{b}")
        nc.vector.tensor_mul(vn, v_ps, rb_b)
        nc.sync.dma_start(desc_v[:, b, :, :], vn)

    # ================= emission schedule ==============================
    # drain engine alternation, ACT-biased
    drain_seq = [nc.scalar, nc.vector] * 16
    dcount = [0]

    def drain_eng():
        e = drain_seq[dcount[0] % len(drain_seq)]
        dcount[0] += 1
        return e

    # --- b0 (arrives in halves) ---
    scores_mms(0, range(0, 4))
    for J, P in [(0, 0), (0, 1), (1, 0), (1, 1)]:
        g = gen_mms(0, J, P)
        gen_drain(0, J, P, g, drain_eng())
    scores_mms(0, range(4, 8))
    exp_op(0, range(0, 8), "e0")
    for J, P in [(2, 0), (2, 1), (3, 0), (3, 1)]:
        g = gen_mms(0, J, P)
        gen_drain(0, J, P, g, drain_eng())
    softmax_tail(0, range(0, 8), nc.gpsimd)

    # --- b1 ---
    scores_mms(1, range(0, 8))
    exp_op(1, range(0, 8), "e1")
    for J in range(NJ):
        vlad_mms(0, J, first=(J == 0), last=(J == NJ - 1))
    for J, P in [(0, 0), (0, 1), (1, 0), (1, 1)]:
        g = gen_mms(1, J, P)
        gen_drain(1, J, P, g, drain_eng())
    softmax_tail(1, range(0, 8), nc.gpsimd)
    centers_mms(0)
    for J, P in [(2, 0), (2, 1), (3, 0), (3, 1)]:
        g = gen_mms(1, J, P)
        gen_drain(1, J, P, g, drain_eng())
    tail_a(0)

    # --- b2 ---
    scores_mms(2, range(0, 8))
    for J in range(NJ):
        vlad_mms(1, J, first=(J == 0), last=(J == NJ - 1))
    exp_op(2, range(0, 8), "e2")
    for J, P in [(0, 0), (0, 1), (1, 0), (1, 1)]:
        g = gen_mms(2, J, P)
        gen_drain(2, J, P, g, drain_eng())
    softmax_tail(2, range(0, 8), nc.gpsimd)
    centers_mms(1)
    tail_a(1)
    for J, P in [(2, 0), (2, 1), (3, 0), (3, 1)]:
        g = gen_mms(2, J, P)
        gen_drain(2, J, P, g, drain_eng())
    for J in range(NJ):
        vlad_mms(2, J, first=(J == 0), last=(J == NJ - 1))
    centers_mms(2)
    tail_a(2)

    # --- b3 (arrives in quarters; chase) ---
    scores_mms(3, range(0, 2))
    g30 = gen_mms(3, 0, 0)
    g31 = gen_mms(3, 0, 1)
    gen_drain(3, 0, 0, g30, nc.scalar)
    gen_drain(3, 0, 1, g31, nc.vector)
    scores_mms(3, range(2, 4))
    g32 = gen_mms(3, 1, 0)
    g33 = gen_mms(3, 1, 1)
    gen_drain(3, 1, 0, g32, nc.scalar)
    gen_drain(3, 1, 1, g33, nc.vector)
    exp_op(3, range(0, 4), "e3a")
    softmax_tail(3, range(0, 4), nc.vector)
    scores_mms(3, range(4, 6))
    g34 = gen_mms(3, 2, 0)
    g35 = gen_mms(3, 2, 1)
    gen_drain(3, 2, 0, g34, nc.scalar)
    gen_drain(3, 2, 1, g35, nc.vector)
    scores_mms(3, range(6, 8))
    g36 = gen_mms(3, 3, 0)
    g37 = gen_mms(3, 3, 1)
    exp_op(3, range(4, 8), "e3b")
    gen_drain(3, 3, 0, g36, nc.scalar)
    gen_drain(3, 3, 1, g37, nc.vector)
    softmax_tail(3, range(4, 8), nc.vector)
    vlad_mms(3, 0, first=True, last=False)
    vlad_mms(3, 1, first=False, last=False)

    # sqrt-table phase for early batches (after the LAST Exp above)
    tail_sqrt(0)
    tail_sqrt(1)
    tail_sqrt(2)
    tail_b(0, nc.scalar)
    tail_b(1, nc.scalar)

    vlad_mms(3, 2, first=False, last=False)
    vlad_mms(3, 3, first=False, last=True)
    centers_mms(3)
    tail_b(2, nc.scalar)
    tail_a(3)
    tail_sqrt(3)
    tail_b(3, nc.scalar)


_NC_CACHE = None


def _build_nc():
    global _NC_CACHE
    if _NC_CACHE is not None:
        return _NC_CACHE
    from contextlib import ExitStack

    nc = bacc.Bacc("TRN2", target_bir_lowering=False, debug=False,
                   num_devices=NCORES)
    xn = nc.dram_tensor("xn", [BC, 128, DC, N], FP8, kind="ExternalInput").ap()
    w = nc.dram_tensor("wt", [128, DC, K], FP8, kind="ExternalInput").ap()
    c = nc.dram_tensor("ct", [K, DC, 128], BF16, kind="ExternalInput").ap()
    out = nc.dram_tensor("desc", [BC, D * K], F32, kind="ExternalOutput").ap()
    with tile.TileContext(nc) as tc, ExitStack() as ctx:
        _netvlad_core(ctx, tc, out, xn, w, c)
    nc.compile()
    _NC_CACHE = nc
    return nc


def _prep_inputs(x, conv_w, centers):
    """Host-side sharding + layout prep (fp8/bf16 cast, d-interleave)."""
    dmap = (4 * np.arange(128)[:, None] + np.arange(DC)[None, :]).reshape(-1)
    wt = np.ascontiguousarray(
        conv_w.T[dmap].reshape(128, DC, K)
    ).astype(NP_FP8)
    ct = np.ascontiguousarray(
        (-centers[dmap].reshape(128, DC, K)).transpose(2, 1, 0)
    ).astype(NP_BF16)
    in_maps = []
    for i in range(NCORES):
        xc = x[i * BC:(i + 1) * BC]
        xn = np.ascontiguousarray(
            xc[:, dmap, :].reshape(BC, 128, DC, N)
        ).astype(NP_FP8)
        in_maps.append({"xn": xn, "wt": wt, "ct": ct})
    return in_maps


def kernel(x, conv_w, centers):
    x = np.ascontiguousarray(x, dtype=np.float32)
    conv_w = np.ascontiguousarray(conv_w, dtype=np.float32)
    centers = np.ascontiguousarray(centers, dtype=np.float32)
    nc = _build_nc()
    in_maps = _prep_inputs(x, conv_w, centers)
    res = run_bass_kernel_spmd(nc, in_maps, core_ids=list(range(NCORES)))
    return np.concatenate([r["desc"] for r in res.results], axis=0)


# revision 8
# speedup vs baseline: 1.4703x; 1.2361x over previous
"""NetVLAD pooling kernel for Trainium2 (Bass/Tile), 8-core data-parallel.

Reference computation (per batch b):
    scores = conv_w @ x[b]                  # [K, N]
    assign = softmax(scores, axis=K)
    vlad   = x[b] @ assign.T - centers * assign.sum(n)   # [D, K]
    vlad  /= max(||vlad||_2 over D, eps)    # intra-norm per cluster column
    desc   = vlad.reshape(D*K) / max(||.||_2, eps)

Shapes: x [32, 512, 1024] f32, conv_w [64, 512], centers [512, 64],
output desc [32, 32768] f32.  Sharding: data-parallel over batch,
4 batches per core; params replicated.

v3 strategy (everything fp8 e4m3 on the PE, DoubleRow perf mode):
  * x ships once in fp8 natural layout (2 MB/core) with d split as
    d = 4p + cc (partition p holds 4 consecutive d rows).  conv_w and
    centers are laid out to match, so the vlad output tile
    [128p, 4cc, 64k] maps to CONTIGUOUS 1 KB runs of the desc row ->
    full-bandwidth output DMA with no transposes.
  * scores^T [n, k] via DoubleRow matmuls (contraction 256/instr,
    0.5 cyc/col): 512 PE cycles per batch.  Softmax over k is a
    free-dim reduce (exp cannot overflow; max subtraction dropped).
    E in bf16; AN in fp8 (multiply on GPSIMD for early batches).
  * x^T for the vlad contraction: batches 0-1 GENERATE it on chip
    (DoubleRow matmul against a block-diagonal fp8 identity transposes
    two d-chunks per instruction; one [128,1024] PSUM->SBUF drain per
    quarter-batch, alternating DVE/ACT).  Batches 2-3 SHIP x^T from
    DRAM in the staged layout, with those DMAs queued AFTER the last
    xn so they ride the tail of the serialized DMA stream without
    delaying any batch's softmax.
  * vlad [d, k] via DoubleRow (staged x^T stationary, AN moving);
    the centers term folds into the same PSUM accumulation as one
    bf16 matmul per d-chunk against diag(asum) (DVE: ident64 * asum);
    asum rides as a tiny DoubleRow ones-matmul.
  * PSUM discipline: on this stack start_tensor_calc marks the whole
    2 KB bank pending-zero, so each bank gets exactly ONE start=True
    matmul per lifetime; later first-writes rely on pending-zero
    overwrite semantics (asum / ssq / rinv-broadcast regions are
    packed into the v bank).
  * intra-norm without transposes: ACT Square drains V -> Vsq bf16,
    PE ones-column matmuls accumulate ssq as a ROW [1, 64], DVE
    reciprocal, ACT Sqrt(q/64) folds the global 1/8 (64 unit columns),
    a 1-partition ones matmul broadcasts rinv to [128, 64], one DVE
    multiply reads V from PSUM against the drained broadcast -> f32 out.
  * ACT tables: Square/Copy live in BOTH act tables, Exp and Sqrt do
    not; all four Exps complete before the single Sqrt-table switch,
    so exactly two table loads.
  * PE warms up on dummy DoubleRow matmuls during the DMA lead-in so
    the p-state ramp completes before real work arrives.
"""

import numpy as np
import ml_dtypes

import concourse.bass as bass
from concourse import bacc
import concourse.mybir as mybir
import concourse.tile as tile
from concourse.bass_utils import run_bass_kernel_spmd
from concourse.masks import make_identity

B, D, K, N = 32, 512, 64, 1024
NCORES = 8
BC = B // NCORES          # batches per core (4)
DC = D // 128             # d chunks (4)
NB = N // 128             # n chunks (8)
NJ = NB // 2              # double-n-chunks (4)
NGEN = 2                  # batches whose x^T is generated on chip
F32 = mybir.dt.float32
BF16 = mybir.dt.bfloat16
FP8 = mybir.dt.float8e4
NP_FP8 = np.dtype(ml_dtypes.float8_e4m3)
NP_BF16 = np.dtype(ml_dtypes.bfloat16)
AF = mybir.ActivationFunctionType
DR = mybir.MatmulPerfMode.DoubleRow


def _netvlad_core(ctx, tc, out, xn, xt, w, c):
    """Emit the per-core tile program.

    out: desc [BC, D*K] f32 DRAM
    xn:  x [BC, 128, DC, N] fp8            (xn[b,p,cc,n] = x[b, 4p+cc, n])
    xt:  staged x^T for batches NGEN..BC-1
         [BC-NGEN, NJ, 128, 2, 2, 2, 128] fp8
         (xt[s,J,np,tp,P,u,i] = x[NGEN+s, 4i+2P+u, 256J+128tp+np])
    w:   conv_w^T  [128, DC, K] fp8        (w[p,cc,k] = conv_w[k, 4p+cc])
    c:   NEGATED centers^T [K, DC, 128] bf16 (c[k,cc,i] = -centers[4i+cc, k])
    """
    nc = tc.nc

    const = ctx.enter_context(tc.tile_pool(name="const", bufs=1))
    xpool = ctx.enter_context(tc.tile_pool(name="xp", bufs=1))
    epool = ctx.enter_context(tc.tile_pool(name="ep", bufs=2))
    apool = ctx.enter_context(tc.tile_pool(name="ap", bufs=BC))
    spool = ctx.enter_context(tc.tile_pool(name="sp", bufs=2))
    stpool = ctx.enter_context(tc.tile_pool(name="st", bufs=2))
    vpool = ctx.enter_context(tc.tile_pool(name="vp", bufs=2))
    opool = ctx.enter_context(tc.tile_pool(name="op", bufs=BC))
    # PSUM: s(2x1) + G(2x2) + v(2x1, smalls packed into the v bank) = 8 banks
    ps_s = ctx.enter_context(tc.tile_pool(name="pss", bufs=2, space="PSUM"))
    ps_g = ctx.enter_context(tc.tile_pool(name="psg", bufs=2, space="PSUM"))
    ps_v = ctx.enter_context(tc.tile_pool(name="psv", bufs=2, space="PSUM"))

    # ---- params + constants ------------------------------------------
    wT = const.tile([128, DC, K], FP8, tag="wT")
    nc.sync.dma_start(wT, w)
    cT = const.tile([K, DC, 128], BF16, tag="cT")
    nc.sync.dma_start(cT, c)

    ibig = const.tile([128, 2, 256], FP8, tag="ibig")
    nc.gpsimd.memset(ibig, 0.0)
    make_identity(nc, ibig[:, 0, 0:128], nomemset=True)
    make_identity(nc, ibig[:, 1, 128:256], nomemset=True)
    id64 = const.tile([64, 64], BF16, tag="id64")
    make_identity(nc, id64)
    ones_col = const.tile([128, 1], BF16, tag="ones_col")
    nc.vector.memset(ones_col, 1.0)
    ones_row = const.tile([1, 128], BF16, tag="ones_row")
    nc.vector.memset(ones_row, 1.0)
    ones2 = const.tile([128, 2, 2], FP8, tag="ones2")
    nc.vector.memset(ones2, 1.0)
    # touch Exp immediately so the 1.3us act-table load overlaps the DMAs
    warm = const.tile([1, 2], F32, tag="warm")
    nc.scalar.activation(warm, ones_row[0:1, 0:2], func=AF.Exp)

    # ---- x loads ------------------------------------------------------
    # Serialized-DMA-stream order: params, xn0 (halves), xn1, xn2, xn3
    # (quarters, tail chase), then shipped x^T (b2 first: its AN is
    # ready long before b3's).  14 DMAs keeps the SP sequencer (~0.65us
    # per DMA) at parity with the transfer stream.
    xns = []
    for b in range(BC):
        xtile = xpool.tile([128, DC, N], FP8, tag="xn", name=f"xn{b}", bufs=BC)
        xns.append(xtile)
    for b, nch in ((0, 2), (1, 1), (2, 1), (3, 4)):
        w_ = N // nch
        for h in range(nch):
            nc.sync.dma_start(
                xns[b][:, :, h * w_:(h + 1) * w_],
                xn[b, :, :, h * w_:(h + 1) * w_],
            )
    # shipped stage tiles: [128, 2tp, 2P, 2u, 128i] per (batch, J)
    stages = {}
    for s in range(BC - NGEN):
        b = NGEN + s
        for Jh in range(2):          # two DMAs per shipped batch (J pairs)
            sh = stpool.tile([128, 2, 2, 2, 2, 128], FP8, tag=f"xq{Jh}",
                             name=f"xq{b}_{Jh}", bufs=BC - NGEN)
            nc.sync.dma_start(
                sh,
                xt[s, 2 * Jh:2 * Jh + 2].rearrange(
                    "j p a b c i -> p j a b c i"),
            )
            for Jo in range(2):
                stages[(b, 2 * Jh + Jo)] = sh[:, Jo]

    desc_v = out.rearrange("b (p cc k) -> p b cc k", p=128, cc=DC, k=K)

    # ---- PE warmup: dummy DoubleRow matmuls during the DMA lead-in ----
    for i in range(10):
        gd = ps_g.tile([128, 2, 2, 256], F32, tag="g", name=f"warmmm{i}")
        for half in range(2):
            nc.tensor.matmul(gd[:, 0, half, :], lhsT=ibig[:, :, 0:128],
                             rhs=ibig, start=True, stop=True, perf_mode=DR)

    # ---- per-batch pieces --------------------------------------------
    s_tiles, Es, ANs, reds, recs = {}, {}, {}, {}, {}
    v_tiles = {}
    vsqs = {}
    q_all = spool.tile([1, BC, K], F32, tag="q", bufs=1)
    rinv_all = spool.tile([1, BC, K], BF16, tag="rinv", bufs=1)

    def scores_mms(b, njs):
        # ONE start=True per PSUM bank per batch (see module docstring).
        if b not in s_tiles:
            s_tiles[b] = (ps_s.tile([128, NB, K], F32, tag="s", name=f"s{b}"),
                          [True])
        s_ps, fresh = s_tiles[b]
        for nj in njs:
            for t in range(2):
                nc.tensor.matmul(
                    s_ps[:, nj, :],
                    lhsT=xns[b][:, 2 * t:2 * t + 2, nj * 128:(nj + 1) * 128],
                    rhs=wT[:, 2 * t:2 * t + 2, :],
                    start=fresh[0], stop=(t == 1), perf_mode=DR,
                    skip_group_check=True,
                )
                fresh[0] = False
        return s_ps

    def gen_mms(b, J):
        """x^T gen for double-chunk J (all 4 d-chunks). Returns PSUM tile."""
        g = ps_g.tile([128, 2, 2, 256], F32, tag="g", name=f"g{b}_{J}")
        for tp in range(2):
            for P in range(2):
                nc.tensor.matmul(
                    g[:, tp, P, :],
                    lhsT=xns[b][:, 2 * P:2 * P + 2,
                                (2 * J + tp) * 128:(2 * J + tp + 1) * 128],
                    rhs=ibig,
                    start=True, stop=True, perf_mode=DR,
                )
        return g

    def gen_drain(b, J, g, eng):
        st = stpool.tile([128, 2, 2, 2, 128], FP8, tag=f"S{J}",
                         name=f"S{b}_{J}", bufs=NGEN)
        if eng is nc.scalar:
            nc.scalar.copy(st, g)
        else:
            nc.vector.tensor_copy(st, g)
        stages[(b, J)] = st

    def exp_op(b, njs):
        s_ps = s_tiles[b][0]
        if b not in Es:
            Es[b] = epool.tile([128, NB, K], BF16, tag="E", name=f"E{b}")
        E = Es[b]
        nj0, nj1 = njs[0], njs[-1] + 1
        nc.scalar.activation(E[:, nj0:nj1, :], s_ps[:, nj0:nj1, :],
                             func=AF.Exp)
        return E

    def softmax_tail(b, njs, mul_eng):
        E = Es[b]
        if b not in reds:
            reds[b] = spool.tile([128, NB], F32, tag="red", name=f"red{b}")
            recs[b] = spool.tile([128, NB], F32, tag="rec", name=f"rec{b}")
            ANs[b] = apool.tile([128, NB, K], FP8, tag="AN", name=f"AN{b}")
        red, rec, AN = reds[b], recs[b], ANs[b]
        nj0, nj1 = njs[0], njs[-1] + 1
        nc.vector.tensor_reduce(
            red[:, nj0:nj1], E[:, nj0:nj1, :],
            axis=mybir.AxisListType.X, op=mybir.AluOpType.add,
        )
        nc.vector.reciprocal(rec[:, nj0:nj1], red[:, nj0:nj1])
        rec_stride = rec.ap[-1][0]
        rec_b = bass.AP(
            tensor=rec.tensor,
            offset=rec.offset + nj0 * rec_stride,
            ap=[rec.ap[0], [rec_stride, nj1 - nj0], [0, K]],
        )
        mul_eng.tensor_mul(AN[:, nj0:nj1, :], E[:, nj0:nj1, :], rec_b)

    def v_views(b):
        vb = v_tiles[b]
        return (vb[:, 0:256].rearrange("p (cc k) -> p cc k", cc=DC),
                vb[0:64, 256:258], vb[0:1, 320:384], vb[:, 384:448])

    def vlad_mms(b, J, first, last):
        if b not in v_tiles:
            v_tiles[b] = ps_v.tile([128, 512], F32, tag="v", name=f"v{b}")
        v_ps, as_ps, _, _ = v_views(b)
        rhs = ANs[b][:, 2 * J:2 * J + 2, :]
        for cc in range(DC):
            nc.tensor.matmul(
                v_ps[:, cc, :],
                lhsT=stages[(b, J)][:, :, cc // 2, cc % 2, :],
                rhs=rhs,
                start=(first and cc == 0), stop=False, perf_mode=DR,
                skip_group_check=True,
            )
        nc.tensor.matmul(
            as_ps, lhsT=rhs, rhs=ones2,
            start=False, stop=last, perf_mode=DR, skip_group_check=True,
        )

    def centers_mms(b):
        v_ps, as_ps, _, _ = v_views(b)
        diag = spool.tile([64, 64], BF16, tag="diag", name=f"diag{b}")
        nc.vector.tensor_scalar_mul(diag, id64, as_ps[:, 0:1])
        for cc in range(DC):
            nc.tensor.matmul(
                v_ps[:, cc, :], lhsT=cT[:, cc, :], rhs=diag,
                start=False, stop=True, skip_group_check=True,
            )

    def tail_a(b):
        """Square-drain V (ACT), column-sum ssq row (PE), reciprocal (DVE)."""
        v_ps, _, ssq, _ = v_views(b)
        vsq = vpool.tile([128, DC, K], BF16, tag="vsq", name=f"vsq{b}")
        nc.scalar.activation(vsq, v_ps, func=AF.Square)
        vsqs[b] = vsq
        for cc in range(DC):
            nc.tensor.matmul(
                ssq, lhsT=ones_col, rhs=vsq[:, cc, :],
                start=False, stop=(cc == DC - 1), skip_group_check=True,
            )
        nc.vector.reciprocal(q_all[:, b, :], ssq)

    def tail_sqrt(bs):
        b0, b1 = bs[0], bs[-1] + 1
        nc.scalar.activation(rinv_all[:, b0:b1, :], q_all[:, b0:b1, :],
                             func=AF.Sqrt, scale=1.0 / 64.0)

    def tail_b(b, cp_eng):
        v_ps, _, _, rb_ps = v_views(b)
        nc.tensor.matmul(rb_ps, lhsT=ones_row, rhs=rinv_all[:, b, :],
                         start=False, stop=True, skip_group_check=True)
        rb = spool.tile([128, K], F32, tag="rb", name=f"rbs{b}")
        if cp_eng is nc.scalar:
            nc.scalar.copy(rb, rb_ps)
        else:
            nc.vector.tensor_copy(rb, rb_ps)
        rb_b = bass.AP(tensor=rb.tensor, offset=rb.offset,
                       ap=[rb.ap[0], [0, DC], [1, K]])
        vn = opool.tile([128, DC, K], F32, tag="vn", name=f"vn{b}")
        nc.vector.tensor_mul(vn, v_ps, rb_b)
        nc.sync.dma_start(desc_v[:, b, :, :], vn)

    # ================= emission schedule ==============================
    # --- b0 (halves) ---
    scores_mms(0, range(0, 4))
    gen_drain(0, 0, gen_mms(0, 0), nc.scalar)
    gen_drain(0, 1, gen_mms(0, 1), nc.vector)
    scores_mms(0, range(4, 8))
    exp_op(0, range(0, 8))
    gen_drain(0, 2, gen_mms(0, 2), nc.scalar)
    gen_drain(0, 3, gen_mms(0, 3), nc.vector)
    softmax_tail(0, range(0, 8), nc.gpsimd)

    # --- b1 ---
    scores_mms(1, range(0, 8))
    exp_op(1, range(0, 8))
    for J in range(NJ):
        vlad_mms(0, J, first=(J == 0), last=(J == NJ - 1))
    gen_drain(1, 0, gen_mms(1, 0), nc.scalar)
    gen_drain(1, 1, gen_mms(1, 1), nc.vector)
    softmax_tail(1, range(0, 8), nc.gpsimd)
    centers_mms(0)
    gen_drain(1, 2, gen_mms(1, 2), nc.scalar)
    gen_drain(1, 3, gen_mms(1, 3), nc.vector)
    tail_a(0)

    # --- b2 (shipped x^T) ---
    scores_mms(2, range(0, 8))
    for J in range(NJ):
        vlad_mms(1, J, first=(J == 0), last=(J == NJ - 1))
    exp_op(2, range(0, 8))
    softmax_tail(2, range(0, 8), nc.gpsimd)
    centers_mms(1)
    tail_a(1)

    # --- b3 (quarters; chase) ---
    scores_mms(3, range(0, 2))
    scores_mms(3, range(2, 4))
    exp_op(3, range(0, 4))
    softmax_tail(3, range(0, 4), nc.vector)
    # b2 vlad chases its shipped stage DMAs
    for J in range(NJ):
        vlad_mms(2, J, first=(J == 0), last=(J == NJ - 1))
    centers_mms(2)
    scores_mms(3, range(4, 6))
    scores_mms(3, range(6, 8))
    exp_op(3, range(4, 8))
    softmax_tail(3, range(4, 8), nc.vector)
    tail_a(2)

    # sqrt-table phase (after the last Exp above): early batches first
    tail_sqrt(range(0, 2))
    tail_b(0, nc.scalar)
    tail_b(1, nc.scalar)

    for J in range(NJ):
        vlad_mms(3, J, first=(J == 0), last=(J == NJ - 1))
    centers_mms(3)
    tail_sqrt(range(2, 3))
    tail_b(2, nc.scalar)
    tail_a(3)
    tail_sqrt(range(3, 4))
    tail_b(3, nc.scalar)


_NC_CACHE = None


def _build_nc():
    global _NC_CACHE
    if _NC_CACHE is not None:
        return _NC_CACHE
    from contextlib import ExitStack

    nc = bacc.Bacc("TRN2", target_bir_lowering=False, debug=False,
                   num_devices=NCORES)
    xn = nc.dram_tensor("xn", [BC, 128, DC, N], FP8, kind="ExternalInput").ap()
    xt = nc.dram_tensor("xt", [BC - NGEN, NJ, 128, 2, 2, 2, 128], FP8,
                        kind="ExternalInput").ap()
    w = nc.dram_tensor("wt", [128, DC, K], FP8, kind="ExternalInput").ap()
    c = nc.dram_tensor("ct", [K, DC, 128], BF16, kind="ExternalInput").ap()
    out = nc.dram_tensor("desc", [BC, D * K], F32, kind="ExternalOutput").ap()
    with tile.TileContext(nc) as tc, ExitStack() as ctx:
        _netvlad_core(ctx, tc, out, xn, xt, w, c)
    nc.compile()
    _NC_CACHE = nc
    return nc


def _prep_inputs(x, conv_w, centers):
    """Host-side sharding + layout prep (fp8/bf16 cast, staging)."""
    wt = np.ascontiguousarray(
        conv_w.T.reshape(128, DC, K)
    ).astype(NP_FP8)
    ct = np.ascontiguousarray(
        (-centers.reshape(128, DC, K)).transpose(2, 1, 0)
    ).astype(NP_BF16)
    in_maps = []
    for i in range(NCORES):
        xc = x[i * BC:(i + 1) * BC]
        xn = np.ascontiguousarray(xc.reshape(BC, 128, DC, N)).astype(NP_FP8)
        # xt[s, J, np, tp, P, u, i] = x[NGEN+s, 4i+2P+u, 256J+128tp+np]
        xs = xc[NGEN:].reshape(BC - NGEN, 128, 2, 2, NJ, 2, 128)
        #      [s, i, P, u, J, tp, np]
        xt = np.ascontiguousarray(
            xs.transpose(0, 4, 6, 5, 2, 3, 1)
        ).astype(NP_FP8)
        in_maps.append({"xn": xn, "xt": xt, "wt": wt, "ct": ct})
    return in_maps


def kernel(x, conv_w, centers):
    x = np.ascontiguousarray(x, dtype=np.float32)
    conv_w = np.ascontiguousarray(conv_w, dtype=np.float32)
    centers = np.ascontiguousarray(centers, dtype=np.float32)
    nc = _build_nc()
    in_maps = _prep_inputs(x, conv_w, centers)
    res = run_bass_kernel_spmd(nc, in_maps, core_ids=list(range(NCORES)))
    return np.concatenate([r["desc"] for r in res.results], axis=0)
